# revision 80
# baseline (speedup 1.0000x reference)
"""Causal multi-head attention (B=4, S=2048, D=1024, H=16) on 8 TRN2 NeuronCores.

Sharding: head-split, zero-collective. Core c handles batch b=c//2 and heads
[8*(c%2), 8*(c%2)+8): it projects Q/K/V for its 8 heads over ALL 2048 tokens
(columns of Wq/Wk/Wv sliced head-wise -- no duplicated projection work),
runs causal attention for those heads over the full sequence, then computes a
PARTIAL output projection against its 512 rows of Wo. The host sums the two
per-batch partials (plus bo + Wo@bv and the 1/256 prescale undo) -- a pure
elementwise unshard, so no on-device collective is needed and all 8 cores run
one identical SPMD graph.

Projections (Q/K/V and the output projection) run in hi/lo-compensated fp8
DoubleRow: x and the weights are split on the host into e4m3 hi + lo parts
(w = hi + lo exactly captures ~14 mantissa bits), and each GEMM evaluates
hi*hi + hi*lo + lo*hi with 256-deep DoubleRow contractions -- 0.75x the bf16
cost per the cost model at bf16-class accuracy. Everything is pre-scaled
(x by 4, weights by 32) to keep the fp8 lo parts out of e4m3's subnormal
floor; the scales fold into the exp scale, the denominator ones-vector, and
one host-side multiply. cT is split hi/lo on the fly by the normalize
(ScalarE casts hi, DVE subtracts the residual).

Attention layout: transposed scores [k, q]. Heads are processed in 4 pairs;
per (pair, k-tile), two N=512 score matmuls fill one two-bank PSUM tile, one
fused ScalarE exp (bias=padding mask) writes bf16 E for both heads; ctx
accumulates per pair (two heads col-packed in one PSUM bank via
tile_position); the softmax denominator accumulates on DVE in bf16 with a
final ones-matmul that both sums across partitions and broadcasts. Queries
stay in natural order; the causal diagonal is handled per 512-token stripe
with a shifted-window triangle mask and a dead-query offset, one strided
instruction covering both heads.

Scheduling: emission interleaves <=2-PSUM-tile projection/output-projection
units between attention k-loop segments so the tensor engine absorbs the
ScalarE exp deficit; normalizes are deferred past the next unit's PSUM->SBUF
copies (DVE head-of-line); startup streams x/weight chunks on both HWDGE
queues; the tail cycles output tiles across all three PSUM pools once
attention has drained.
"""

import os
import sys

sys.path.insert(0, "/opt/trn_rl_repo")

import numpy as np
import ml_dtypes

import concourse.bass as bass
import concourse.bacc as bacc
import concourse.tile as tile
from concourse import mybir
from concourse.bass_utils import run_bass_kernel_spmd

B, S, D, H = 4, 2048, 1024, 16
HD = D // H  # 64
P = 128
KC = D // P      # 8 contraction chunks
NP = 4           # head-pair chunks of 128 output dims (512 dims per core)
QW = 512         # query stripe width
NKT = S // P     # 16 key tiles
NEG = -1e30
BF16 = mybir.dt.bfloat16
F32 = mybir.dt.float32
F8 = mybir.dt.float8e4
NPBF16 = ml_dtypes.bfloat16
NPF8 = ml_dtypes.float8_e4m3
# prescales keep every fp8 hi/lo encoding out of e4m3's subnormal floor
# (W sigma=0.02 -> 0.64); they fold into EXPSCL, the ones vector, and one
# host-side multiply, so no extra device work is needed.
WSCALE = 32.0   # weight prescale (wq/wk/wv/wo)
XSCALE = 4.0    # activation prescale (x)
CSCALE = 8.0    # scale at which normalized cT is produced for its fp8 split
# exp scale folds away the q'.k' = (XSCALE*WSCALE)^2 overall scaling
EXPSCL = 0.125 / (XSCALE * WSCALE) ** 2
# v' = XSCALE*WSCALE*v accumulates into ctx'; a (XSCALE*WSCALE/CSCALE)-valued
# ones vector makes den_ps carry the matching scale so ctmp = CSCALE*ctx
ONESV = XSCALE * WSCALE / CSCALE


def _build():
    nc = bacc.Bacc()

    # all projections run hi/lo-compensated fp8 DoubleRow (4 dr-chunks of
    # 256 contraction x 3 terms = 0.75x the bf16 PE cost)
    xh = nc.declare_dram_parameter("xh", [P, KC, S], F8, isOutput=False)
    xl = nc.declare_dram_parameter("xl", [P, KC, S], F8, isOutput=False)
    wqh = nc.declare_dram_parameter("wqh", [P, KC, NP * P], F8, isOutput=False)
    wql = nc.declare_dram_parameter("wql", [P, KC, NP * P], F8, isOutput=False)
    wkh = nc.declare_dram_parameter("wkh", [P, KC, NP * P], F8, isOutput=False)
    wkl = nc.declare_dram_parameter("wkl", [P, KC, NP * P], F8, isOutput=False)
    wvh = nc.declare_dram_parameter("wvh", [P, KC, NP * P], F8, isOutput=False)
    wvl = nc.declare_dram_parameter("wvl", [P, KC, NP * P], F8, isOutput=False)
    woh = nc.declare_dram_parameter("woh", [P, NP, D], F8, isOutput=False)
    wol = nc.declare_dram_parameter("wol", [P, NP, D], F8, isOutput=False)
    bqp = nc.declare_dram_parameter("bqp", [P, NP], F32, isOutput=False)
    bkp = nc.declare_dram_parameter("bkp", [P, NP], F32, isOutput=False)
    pad = nc.declare_dram_parameter("pad", [P, NKT], F32, isOutput=False)
    tri = nc.declare_dram_parameter("tri", [P, 896], BF16, isOutput=False)
    out = nc.declare_dram_parameter("out", [S, D], BF16, isOutput=True)

    from contextlib import ExitStack

    with tile.TileContext(nc) as tc, ExitStack() as ctx:
        wpool = ctx.enter_context(tc.tile_pool(name="wpool", bufs=1))
        xpool = ctx.enter_context(tc.tile_pool(name="xpool", bufs=3))
        bigpool = ctx.enter_context(tc.tile_pool(name="bigpool", bufs=1))
        epool = ctx.enter_context(tc.tile_pool(name="epool", bufs=12))
        dpool = ctx.enter_context(tc.tile_pool(name="dpool", bufs=8))
        spool = ctx.enter_context(tc.tile_pool(name="spool", bufs=7))
        pp_acc = ctx.enter_context(tc.tile_pool(name="pp_acc", bufs=2, space="PSUM"))
        pp_sc = ctx.enter_context(tc.tile_pool(name="pp_sc", bufs=2, space="PSUM"))
        pp_ctx = ctx.enter_context(tc.tile_pool(name="pp_ctx", bufs=2, space="PSUM"))

        # ---- constants into SBUF ----
        wf8_s = {(part, hl): wpool.tile([P, KC, NP * P], F8,
                                        tag=f"w{part}{hl}",
                                        name=f"w{part}{hl}_s")
                 for part in "qkv" for hl in "hl"}
        wo8_s = (wpool.tile([P, NP, D], F8, tag="woh", name="woh_s"),
                 wpool.tile([P, NP, D], F8, tag="wol", name="wol_s"))
        bq_s = wpool.tile([P, NP], F32, tag="bq")
        bk_s = wpool.tile([P, NP], F32, tag="bk")
        pad_s = wpool.tile([P, NKT], F32, tag="pad")
        tri_s = wpool.tile([P, 896], BF16, tag="tri")
        ones_s = wpool.tile([P, HD], BF16, tag="ones")

        # ---- big persistent activations ----
        qT_s = bigpool.tile([P, NP, S], BF16, tag="qT")   # [pairdim, pair, q]
        kT_s = bigpool.tile([P, NP, S], BF16, tag="kT")   # [pairdim, pair, k]
        v_s = bigpool.tile([P, NKT, NP * P], BF16, tag="v")  # [k in tile, ktile, d]
        cT8h_s = bigpool.tile([P, NP, S], F8, tag="cT8h")  # [pairdim, pair, q]
        cT8l_s = bigpool.tile([P, NP, S], F8, tag="cT8l")

        def load_xt(st):
            ssl = slice(st * QW, (st + 1) * QW)
            xh_t = xpool.tile([P, KC, QW], F8, tag="xh")
            xl_t = xpool.tile([P, KC, QW], F8, tag="xl")
            nc.sync.dma_start(xh_t[:], xh[:, :, ssl])
            nc.sync.dma_start(xl_t[:], xl[:, :, ssl])
            return xh_t, xl_t

        def proj(st, xt_t, part, tiles):
            """Project tokens [st*512, (st+1)*512) of this core's 8 heads.
            Emits only the listed tiles so segments can interleave with
            attention k-loops (keep units <= 2 tiles: pp_acc has 2 bufs).
            Hi/lo-compensated fp8 DoubleRow: 4 dr-chunks of 256 contraction
            x 3 terms = 0.75x the bf16 PE cost."""
            ssl = slice(st * QW, (st + 1) * QW)
            DR = mybir.MatmulPerfMode.DoubleRow
            if part == "v":
                # transposed: [token, dim] so keys sit on partitions
                for sub in tiles:
                    xsl = slice(sub * P, (sub + 1) * P)
                    ps = pp_acc.tile([P, NP * P], F32, tag="acc")
                    xh_t, xl_t = xt_t
                    vh, vl = wf8_s[("v", "h")], wf8_s[("v", "l")]
                    for half in range(2):
                        dsl = slice(half * 256, (half + 1) * 256)
                        n = 0
                        # term order matches the startup DMA arrival order
                        # (xh, wvh, xl, wvl) so stripe-0 tiles never wait
                        for xtt, wtt in ((xh_t, vh), (xl_t, vh),
                                         (xh_t, vl)):
                            for t in range(4):
                                nc.tensor.matmul(
                                    ps[:, dsl],
                                    lhsT=xtt[:, 2 * t:2 * t + 2, xsl],
                                    rhs=wtt[:, 2 * t:2 * t + 2, dsl],
                                    start=(n == 0), stop=(n == 11),
                                    perf_mode=DR)
                                n += 1
                    # alternate copy engine so neither the DVE nor the ACT
                    # queue backlog stalls the pp_acc rotation
                    if sub % 2 == 0:
                        nc.vector.tensor_copy(out=v_s[:, st * 4 + sub, :],
                                              in_=ps[:])
                    else:
                        nc.scalar.activation(
                            v_s[:, st * 4 + sub, :], ps[:],
                            mybir.ActivationFunctionType.Identity,
                            bias=0.0, scale=1.0)
                return
            b_s, dst = (bq_s, qT_s) if part == "q" else (bk_s, kT_s)
            for m in tiles:
                msl = slice(m * P, (m + 1) * P)
                ps = pp_acc.tile([P, QW], F32, tag="acc")
                xh_t, xl_t = xt_t
                wh, wl = wf8_s[(part, "h")], wf8_s[(part, "l")]
                for half in range(2):
                    tsl = slice(half * 256, (half + 1) * 256)
                    n = 0
                    for wtt, xtt in ((wh, xh_t), (wh, xl_t), (wl, xh_t)):
                        for t in range(4):
                            nc.tensor.matmul(
                                ps[:, tsl],
                                lhsT=wtt[:, 2 * t:2 * t + 2, msl],
                                rhs=xtt[:, 2 * t:2 * t + 2, tsl],
                                start=(n == 0), stop=(n == 11),
                                perf_mode=DR)
                            n += 1
                if m % 2 == 0:
                    nc.vector.tensor_scalar_add(dst[:, m, ssl], ps[:],
                                                b_s[:, m:m + 1])
                else:
                    nc.scalar.activation(dst[:, m, ssl], ps[:],
                                         mybir.ActivationFunctionType.Identity,
                                         bias=b_s[:, m:m + 1], scale=1.0)

        attn_state = {}

        def attn(st, prs, jlo=0, jhi=None):
            """K-loop segment [jlo, jhi) for query stripe st, head pairs prs
            (a group of 2), interleaved across the two pairs. At the end of
            the k-loop the denominator matmuls are emitted; the DVE
            reciprocal/normalize is deferred to finish() so it queues behind
            the next segment's PSUM->SBUF copies instead of ahead of them."""
            nkt = 4 * (st + 1)
            if jhi is None:
                jhi = nkt
            if jlo == 0:
                for pr in prs:
                    attn_state[(st, pr)] = (
                        pp_ctx.tile([P, QW], F32, tag="ctx",
                                    name=f"ctx_s{st}p{pr}"),
                        dpool.tile([P, 2, QW], BF16, tag="dacc",
                                   name=f"dacc_s{st}p{pr}"))
            ctx_ps = {pr: attn_state[(st, pr)][0] for pr in prs}
            dacc = {pr: attn_state[(st, pr)][1] for pr in prs}
            e0 = {}
            for j in range(jlo, jhi):
                m = j
                ksl = slice(m * P, (m + 1) * P)
                diag = m >= 4 * st
                off = (m - 4 * st) * P if diag else 0
                w = QW - off
                qsub = slice(st * QW + off, (st + 1) * QW)
                for pr in prs:
                    # scores for both heads of the pair in adjacent banks
                    sc = pp_sc.tile([P, 2, QW], F32, tag="sc")
                    for q_i in range(2):
                        lo = q_i * HD
                        nc.tensor.matmul(
                            sc[:, q_i, off:],
                            lhsT=kT_s[lo:lo + HD, pr, ksl],
                            rhs=qT_s[lo:lo + HD, pr, qsub],
                            start=True, stop=True, tile_position=(lo, 0))
                    e = epool.tile([P, 2, QW], BF16, tag="e")
                    # one strided instruction covers both heads' valid ranges
                    nc.scalar.activation(e[:, :, off:], sc[:, :, off:],
                                         mybir.ActivationFunctionType.Exp,
                                         bias=pad_s[:, m:m + 1],
                                         scale=EXPSCL)
                    if diag:
                        nc.vector.tensor_tensor(
                            e[:, :, off:], e[:, :, off:],
                            tri_s[:, None, 384:384 + w].broadcast_to(
                                (P, 2, w)),
                            mybir.AluOpType.mult)
                    if j == 0 and st > 0:
                        # defer: j0+j1 fuse into one add (both full width
                        # for stripes >= 1, where j0/j1 are off-diagonal)
                        e0[pr] = e
                    elif j == 0:
                        nc.vector.tensor_scalar_add(dacc[pr][:], e[:], 0.0)
                    elif j == 1 and st > 0:
                        nc.vector.tensor_tensor(dacc[pr][:], e0.pop(pr)[:],
                                                e[:], mybir.AluOpType.add)
                    else:
                        nc.vector.tensor_tensor(dacc[pr][:, :, off:],
                                                dacc[pr][:, :, off:],
                                                e[:, :, off:],
                                                mybir.AluOpType.add)
                    st_, sp_ = (j == 0), (j == nkt - 1)
                    for q_i in range(2):
                        h = 2 * pr + q_i
                        lo = q_i * HD
                        nc.tensor.matmul(
                            ctx_ps[pr][lo:lo + HD, off:],
                            lhsT=v_s[:, m, h * HD:(h + 1) * HD],
                            rhs=e[:, q_i, off:],
                            start=st_, stop=sp_, tile_position=(0, lo),
                            skip_group_check=True)
            if jhi < nkt:
                return
            for pr in prs:
                attn_state[(st, pr)] = (ctx_ps[pr], dacc[pr])

        def finish(st, prs):
            """Deferred softmax normalize (DVE) for completed k-loops, plus
            the hi/lo fp8 split of cT that feeds the DoubleRow output
            projection (hi cast rides ScalarE, residual on DVE)."""
            qsl = slice(st * QW, (st + 1) * QW)
            st_ = {}
            # the denominator matmuls live here, after the boundary filler
            # unit, so they never wait on the ACT-lagged dacc tail; then
            # software-pipelined recips/mults/casts/subs across pairs
            for pr in prs:
                ctx_ps, dacc = attn_state.pop((st, pr))
                den_ps = pp_sc.tile([P, 2 * QW], F32, tag="sc",
                                    name=f"den_s{st}p{pr}")
                for q_i in range(2):
                    lo = q_i * HD
                    nc.tensor.matmul(
                        den_ps[lo:lo + HD, 0:QW],
                        lhsT=ones_s[:],
                        rhs=dacc[:, q_i, :],
                        start=True, stop=True, tile_position=(0, lo),
                        skip_group_check=True)
                rden = spool.tile([P, QW], F32, tag="rden")
                nc.vector.reciprocal(rden[:], den_ps[:, 0:QW])
                st_[pr] = (ctx_ps, rden)
            for pr in prs:
                ctx_ps, rden = st_[pr]
                ctmp = spool.tile([P, QW], F32, tag="ctmp")
                nc.vector.tensor_tensor(ctmp[:], ctx_ps[:],
                                        rden[:], mybir.AluOpType.mult)
                nc.scalar.activation(cT8h_s[:, pr, qsl], ctmp[:],
                                     mybir.ActivationFunctionType.Identity,
                                     bias=0.0, scale=1.0)
                st_[pr] = ctmp
            for pr in prs:
                nc.vector.tensor_tensor(cT8l_s[:, pr, qsl], st_[pr][:],
                                        cT8h_s[:, pr, qsl],
                                        mybir.AluOpType.subtract)

        def op_ps_cycled(i):
            """Rotate the tail oproj PSUM tiles across all three pools --
            attention is over, so pp_sc/pp_ctx slots are free and the
            rotation depth triples."""
            k = i % 3
            if k == 0:
                return pp_acc.tile([P, 512], F32, tag="acc", name="ops_acc")
            if k == 1:
                t2 = pp_sc.tile([P, 2, QW], F32, tag="sc", name="ops_sc")
                return t2[:, 0, :]
            return pp_ctx.tile([P, QW], F32, tag="ctx", name="ops_ctx")

        def oproj(st, tts, act_copy=False, cycle_pools=False):
            """Partial output projection (this core's 512 ctx dims) for the
            listed 128-token tiles of stripe st, in compensated fp8
            DoubleRow. Tail units copy via ScalarE (act_copy) -- the exp
            stream is drained by then and DVE still holds the last
            normalizes."""
            DR = mybir.MatmulPerfMode.DoubleRow
            woh, wol = wo8_s
            psn = 0
            for tt in tts:
                osl = slice(st * QW + tt * P, st * QW + (tt + 1) * P)
                for dt in range(2):
                    dsl = slice(dt * 512, (dt + 1) * 512)
                    if cycle_pools:
                        ps = op_ps_cycled(psn)
                        psn += 1
                    else:
                        ps = pp_acc.tile([P, 512], F32, tag="acc")
                    for half in range(2):
                        hsl = slice(dt * 512 + half * 256,
                                    dt * 512 + (half + 1) * 256)
                        n = 0
                        # kc-chunk-major: chunk 0 reads only pairs 0-1's cT,
                        # so these matmuls overlap the final finish() chain
                        for t in range(2):
                            for ctt, wtt in ((cT8h_s, woh), (cT8h_s, wol),
                                             (cT8l_s, woh)):
                                nc.tensor.matmul(
                                    ps[:, half * 256:(half + 1) * 256],
                                    lhsT=ctt[:, 2 * t:2 * t + 2, osl],
                                    rhs=wtt[:, 2 * t:2 * t + 2, hsl],
                                    start=(n == 0), stop=(n == 5),
                                    perf_mode=DR)
                                n += 1
                    ob = spool.tile([P, 512], BF16, tag="outsb")
                    if act_copy:
                        # tail: split across DVE and ScalarE (exp stream is
                        # drained, both queues shallow)
                        nc.vector.tensor_copy(out=ob[:, 0:256],
                                              in_=ps[:, 0:256])
                        nc.scalar.activation(
                            ob[:, 256:512], ps[:, 256:512],
                            mybir.ActivationFunctionType.Identity,
                            bias=0.0, scale=1.0)
                    else:
                        nc.vector.tensor_copy(out=ob[:], in_=ps[:])
                    nc.sync.dma_start(out[osl, dsl], ob[:])

        # startup: two HWDGE queues (SP + ACT) in parallel, first chunks
        # split so the first V-proj matmuls start after ~1.7us. DMA triggers
        # are emitted before the warm-up exp so the ACT-queue transfers
        # aren't serialized behind the ~1.3us ACT table load.
        xh0 = xpool.tile([P, KC, QW], F8, tag="xh")
        xl0 = xpool.tile([P, KC, QW], F8, tag="xl")
        nc.sync.dma_start(xh0[:, 0:4, :], xh[:, 0:4, 0:QW])
        nc.scalar.dma_start(wf8_s[("v", "h")][:, 0:4, :], wvh[:, 0:4, :])
        nc.sync.dma_start(xh0[:, 4:KC, :], xh[:, 4:KC, 0:QW])
        nc.scalar.dma_start(wf8_s[("v", "h")][:, 4:KC, :], wvh[:, 4:KC, :])
        nc.sync.dma_start(xl0[:], xl[:, :, 0:QW])
        nc.scalar.dma_start(wf8_s[("v", "l")][:], wvl[:])
        nc.sync.dma_start(wf8_s[("q", "h")][:], wqh[:])
        nc.scalar.dma_start(wf8_s[("q", "l")][:], wql[:])
        nc.sync.dma_start(wf8_s[("k", "h")][:], wkh[:])
        nc.scalar.dma_start(wf8_s[("k", "l")][:], wkl[:])
        nc.sync.dma_start(bq_s[:], bqp[:])
        nc.scalar.dma_start(bk_s[:], bkp[:])
        nc.scalar.dma_start(pad_s[:], pad[:])
        nc.scalar.dma_start(tri_s[:], tri[:])
        xt0 = (xh0, xl0)
        nc.vector.memset(ones_s[:], ONESV)
        # touch Exp once at t~0 so the ~1.3us ACT table load happens inside
        # the startup DMA shadow, not at the first real softmax
        warm_s = wpool.tile([P, 1], F32, tag="warm")
        nc.vector.memset(warm_s[:], 0.0)
        nc.scalar.activation(warm_s[:], warm_s[:],
                             mybir.ActivationFunctionType.Exp, scale=1.0)
        proj(0, xt0, "v", [0, 1, 2, 3])
        proj(0, xt0, "q", [0, 1, 2, 3])
        proj(0, xt0, "k", [0, 1, 2, 3])
        xt1 = load_xt(1)
        nc.sync.dma_start(wo8_s[0][:], woh[:])
        nc.scalar.dma_start(wo8_s[1][:], wol[:])
        # stripe 0 (nkt=4); finishes go after one filler unit so the
        # filler's PSUM->SBUF copies beat the normalize chain in the DVE queue
        attn(0, (0, 1))
        proj(1, xt1, "v", [0, 1])
        finish(0, (0, 1))
        attn(0, (2, 3))
        proj(1, xt1, "v", [2, 3])
        finish(0, (2, 3))
        proj(1, xt1, "q", [0, 1])
        proj(1, xt1, "q", [2, 3])
        proj(1, xt1, "k", [0, 1])
        proj(1, xt1, "k", [2, 3])
        xt2 = load_xt(2)
        # stripe 1 (nkt=8)
        attn(1, (0, 1), 0, 5)
        proj(2, xt2, "v", [0])
        attn(1, (0, 1), 5, 8)
        proj(2, xt2, "v", [1])
        finish(1, (0, 1))
        attn(1, (2, 3), 0, 5)
        proj(2, xt2, "v", [2])
        attn(1, (2, 3), 5, 8)
        proj(2, xt2, "v", [3])
        finish(1, (2, 3))
        proj(2, xt2, "q", [0, 1])
        proj(2, xt2, "q", [2, 3])
        proj(2, xt2, "k", [0, 1])
        proj(2, xt2, "k", [2, 3])
        xt3 = load_xt(3)
        # stripe 2 (nkt=12)
        attn(2, (0, 1), 0, 4)
        proj(3, xt3, "v", [0])
        attn(2, (0, 1), 4, 8)
        proj(3, xt3, "v", [1])
        attn(2, (0, 1), 8, 12)
        oproj(0, [0])
        finish(2, (0, 1))
        attn(2, (2, 3), 0, 4)
        proj(3, xt3, "v", [2])
        attn(2, (2, 3), 4, 8)
        proj(3, xt3, "v", [3])
        attn(2, (2, 3), 8, 12)
        oproj(0, [1])
        finish(2, (2, 3))
        proj(3, xt3, "q", [0, 1])
        proj(3, xt3, "q", [2, 3])
        proj(3, xt3, "k", [0, 1])
        proj(3, xt3, "k", [2, 3])
        # stripe 3 (nkt=16): keep >=4 k-tiles between oproj fillers so each
        # filler's PSUM copy (queued behind ACT-paced dacc ops on DVE) lands
        # before the next filler's rotation needs the bank back
        attn(3, (0, 1), 0, 4)
        oproj(0, [2])
        attn(3, (0, 1), 4, 8)
        oproj(0, [3])
        attn(3, (0, 1), 8, 12)
        oproj(1, [0])
        attn(3, (0, 1), 12, 16)
        oproj(1, [1])
        finish(3, (0, 1))
        attn(3, (2, 3), 0, 3)
        oproj(1, [2])
        attn(3, (2, 3), 3, 6)
        oproj(1, [3])
        attn(3, (2, 3), 6, 9)
        oproj(2, [0])
        attn(3, (2, 3), 9, 12)
        oproj(2, [1])
        attn(3, (2, 3), 12, 14)
        oproj(2, [2])
        attn(3, (2, 3), 14, 16)
        oproj(2, [3])
        finish(3, (2, 3))
        oproj(3, [0, 1], act_copy=True, cycle_pools=True)
        oproj(3, [2, 3], act_copy=True, cycle_pools=True)

    nc.compile()
    return nc


def _core_inputs(c, x, padding_mask, Wq, bq, Wk, bk, Wv, bv, Wo, bo):
    b, hh = c // 2, c % 2
    hsl = slice(hh * 512, (hh + 1) * 512)

    # [P, KC, S] = [contraction-part, contraction-chunk, token],
    # hi/lo fp8 split for the DoubleRow projections
    xfull = np.ascontiguousarray(
        (x[b].T * XSCALE).reshape(KC, P, S).transpose(1, 0, 2))
    xhp = xfull.astype(NPF8)
    xlp = (xfull - xhp.astype(np.float32)).astype(NPF8)

    def wl(W):
        # 32x prescale keeps fp8 encodings out of e4m3's subnormal floor;
        # folded back via EXPSCL / the 32-valued ones vector.
        return np.ascontiguousarray(
            (W.T * WSCALE).reshape(KC, P, D).transpose(1, 0, 2)[:, :, hsl]
        ).astype(np.float32)

    def split8(Wf):
        hi = Wf.astype(NPF8)
        lo = (Wf - hi.astype(np.float32)).astype(NPF8)
        return np.ascontiguousarray(hi), np.ascontiguousarray(lo)

    wqf, wkf, wvf = wl(Wq), wl(Wk), wl(Wv)
    wqh, wql = split8(wqf)
    wkh, wkl = split8(wkf)
    wvh, wvl = split8(wvf)

    wof = np.ascontiguousarray(
        (Wo.T * WSCALE).reshape(KC, P, D).transpose(1, 0, 2)
        [:, 4 * hh:4 * hh + 4, :]).astype(np.float32)
    woh, wol = split8(wof)

    bqp = np.ascontiguousarray(
        bq[hsl].reshape(NP, P).T * (XSCALE * WSCALE)).astype(np.float32)
    bkp = np.ascontiguousarray(
        bk[hsl].reshape(NP, P).T * (XSCALE * WSCALE)).astype(np.float32)

    # pad bias [P, 16]: 0 where the key is unpadded, else -1e30
    padb = np.where(padding_mask[b].reshape(NKT, P).T, 0.0, NEG).astype(
        np.float32)

    # tri [P, 896]: all diagonal shift patterns are windows of one function:
    # tri[p, u] = (p <= u - 384); shift t's mask over the valid query range
    # [t*128, 512) is the slice [384, 384 + 512 - t*128).
    kk = np.arange(P)[:, None]
    uu = np.arange(896)[None, :]
    trib = (kk <= uu - 384).astype(NPBF16)

    return {"xh": xhp, "xl": xlp,
            "wqh": wqh, "wql": wql, "wkh": wkh, "wkl": wkl,
            "wvh": wvh, "wvl": wvl, "woh": woh, "wol": wol,
            "bqp": bqp, "bkp": bkp,
            "pad": np.ascontiguousarray(padb),
            "tri": np.ascontiguousarray(trib)}


_NC_CACHE = {}


def kernel(x, padding_mask, Wq, bq, Wk, bk, Wv, bv, Wo, bo):
    x = np.asarray(x, np.float32)
    padding_mask = np.asarray(padding_mask, bool)
    args = [np.asarray(a, np.float32) for a in (Wq, bq, Wk, bk, Wv, bv, Wo, bo)]

    if "nc" not in _NC_CACHE:
        _NC_CACHE["nc"] = _build()
    nc = _NC_CACHE["nc"]

    in_maps = [_core_inputs(c, x, padding_mask, *args) for c in range(8)]

    trace = bool(int(os.environ.get("KERNEL_TRACE", "0")))
    try:
        res = run_bass_kernel_spmd(nc, in_maps, core_ids=list(range(8)), trace=trace)
    except ModuleNotFoundError:
        # NTFF profiling hook unavailable in this environment
        res = run_bass_kernel_spmd(nc, in_maps, core_ids=list(range(8)))
    if trace and res.exec_time_ns is not None:
        print(f"HW exec time: {res.exec_time_ns} ns")
        _NC_CACHE["exec_time_ns"] = res.exec_time_ns

    # softmax weights sum to 1, so the V bias passes through attention
    # unchanged and folds into the output bias: out += Wo @ bv + bo
    bo2 = (args[7] + args[6] @ args[5]).astype(np.float32)
    full = np.empty((B, S, D), np.float32)
    for b in range(B):
        np.add(res.results[2 * b]["out"], res.results[2 * b + 1]["out"],
               out=full[b])
        # undo the cT (CSCALE) and Wo (WSCALE) prescales
        full[b] *= np.float32(1.0 / (CSCALE * WSCALE))
        full[b] += bo2
    return full


if __name__ == "__main__":
    rng = np.random.default_rng(0)
    x = rng.standard_normal((B, S, D), dtype=np.float32)
    lengths = rng.integers(S // 2, S + 1, size=(B,))
    pm = np.arange(S)[None, :] < lengths[:, None]
    std = 0.02
    ws = {n: (rng.standard_normal((D, D), dtype=np.float32) * std)
          for n in ("Wq", "Wk", "Wv", "Wo")}
    z = np.zeros((D,), np.float32)
    out = kernel(x, pm, ws["Wq"], z, ws["Wk"], z, ws["Wv"], z, ws["Wo"], z)
    print(out.shape, out.dtype, np.abs(out).mean())
